# revision 21
# baseline (speedup 1.0000x reference)
"""Trainium2 Bass kernel for nn_Decoder_58514634440787 (histogram_binning).

Piecewise-linear decoder: y[b, s] = interp of (segment_x, segment_y) knots
evaluated at the uniform pixel grid t_s = (s+1)/S, S = 196608, B = 8.

The output along the pixel axis is piecewise linear with at most 33 knots
per batch.  Pixels are sharded across 8 cores (24576 each) and laid out
on-chip as [128 partitions = 8 batches x 16 rows, 1536 pixels].  Each
1536-pixel row intersects at most a couple of knots.  The host converts the
tiny [8, 33] knot tensors into per-row line parameters in *column space*
(f = 0..1535 within the row; t = (s0 + f + 1)/S is folded into slope and
intercept in float64), so the device only needs:

    f    = iota along the free axis                  (GPSIMD, exact in f32)
    out  = aB[p]*f + bB[p]                           (base line)
    patch left  half where f <  colL[p] with line (aL, bL)
    patch right half where f >= colR[p] with line (aR, bR)

Masks are integer compares against host-computed breakpoint columns (exact:
the host does searchsorted on the exact f32 grid, the same predicate the
reference evaluates), and line evaluation runs on the Scalar/Vector engines
as per-partition scale+bias (the Scalar engine fuses f*a+b in one rounding).
No big input tensor: the only DMA in is a small transposed parameter block
[rows, 128+rows] (32 wide descriptors instead of 128 tiny ones, identity
appended) which the otherwise-idle TensorEngine un-transposes via an
identity matmul.  Output stores are split across both HWDGE queues
(sync + scalar) to double store bandwidth.

The number of patch slots per half-row adapts to the data ((1,1) for the
benchmark inputs); each additional slot adds one mask + line + predicated
copy.  Correct for any input with at most ~20 breakpoint slots per
768-pixel half-row (beyond that _host_prep asserts).

Inputs are the full [8, 33] knot tensors; sharding/gather happens here.
Measured on 8 axon trn2 cores: HW exec time ~18.3 us, relative error
(norm) 1.7e-7, max elementwise 2.2e-4 vs the jax-cpu reference.
"""

import numpy as np

S = 196608
B = 8
W = 1536              # pixels per partition row
RPB = 16              # rows per batch per core
P = 128               # partitions = B * RPB
NCORES = 8
PIX_PER_CORE = RPB * W  # 24576
HALF = W // 2

_t_grid = None          # f32 [S] exact (s+1)/S
_compiled = {}          # (n_left, n_right) -> nc


def _get_grid():
    global _t_grid
    if _t_grid is None:
        _t_grid = (np.arange(1, S + 1, dtype=np.float64) / S).astype(np.float32)
    return _t_grid


def _fix_x_order(sx, sy):
    """Running max of x along the segment axis, y carried from the position
    achieving the max (ties keep the later entry). Matches reference."""
    x = sx.copy()
    y = sy.copy()
    for b in range(sx.shape[0]):
        cx, cy = sx[b, 0], sy[b, 0]
        for i in range(sx.shape[1]):
            if sx[b, i] >= cx:
                cx, cy = sx[b, i], sy[b, i]
            x[b, i] = cx
            y[b, i] = cy
    return x, y


def _host_prep(segment_x, segment_y):
    """Returns (pT_per_core, (n_left, n_right)).

    pT_per_core: [32, 128] f32; row j holds parameter j for all 128
    partitions.  Parameters per partition row (iota-column space):
      [aB, bB, (colL_j, aL_j, bL_j)..., (colR_j, aR_j, bR_j)...]
    Left slots are ordered latest-breakpoint-first; right slots
    earliest-first.  line(f) = a*f + b  with  a = ratio/S  and
    b = ratio*((s0+1)/S - x_m) + y_m  computed in float64.
    """
    t_grid = _get_grid()
    sx = np.asarray(segment_x, dtype=np.float32)
    sy = np.asarray(segment_y, dtype=np.float32)
    x, y = _fix_x_order(sx, sy)

    gaps = x[:, 1:] - x[:, :-1]
    div = np.where(gaps == 0.0, np.float32(0.0001), gaps).astype(np.float32)
    a = ((y[:, 1:] - y[:, :-1]) / div).astype(np.float32)          # [B, 32]
    a64 = a.astype(np.float64)
    x64 = x.astype(np.float64)
    y64 = y.astype(np.float64)

    # First pixel index s with t_s >= x_n, for binning knots n = 1..31.
    # searchsorted on the exact f32 grid == the reference's f32 compares.
    k = np.stack([np.searchsorted(t_grid, x[b, 1:32], side='left')
                  for b in range(B)])                               # [B, 31]

    # per (batch, global row): breakpoints, dedup by pixel keeping largest n
    rows = [[dict() for _ in range(NCORES * RPB)] for _ in range(B)]
    for b in range(B):
        for n in range(31):
            kk = int(k[b, n])
            if kk < S:
                rows[b][kk // W][kk % W] = n + 1   # knot index 1..31
    ks = [np.sort(k[b]) for b in range(B)]

    def seg(b, s):
        # segment index at pixel s = number of breakpoints with k <= s
        return int(np.searchsorted(ks[b], s, side='right'))

    def line(b, m, s0):
        # (slope, intercept) in local column space for segment m of batch b,
        # for a span starting at global pixel s0 (f local to that span)
        aa = a64[b, m]
        bb = aa * ((s0 + 1) / S - x64[b, m]) + y64[b, m]
        return (np.float32(aa / S), np.float32(bb))

    n_left = n_right = 0
    per_row = []
    for c in range(NCORES):
        core_rows = []
        for b in range(B):
            for r in range(RPB):
                g = c * RPB + r
                s0 = c * PIX_PER_CORE + r * W
                bps = sorted(rows[b][g].items())   # [(col, knot_n)...]
                left = [(col, n) for col, n in bps if col < HALF]
                right = [(col, n) for col, n in bps if col >= HALF]
                n_left = max(n_left, len(left))
                n_right = max(n_right, len(right))
                mb = seg(b, s0 + HALF - 1)
                baseL = line(b, mb, s0)             # f in [0, HALF)
                baseR = line(b, mb, s0 + HALF)[1]   # same slope, right span
                lslots = []
                for col, n in sorted(left, reverse=True):
                    m_prev = seg(b, s0 + col - 1)
                    lslots.append((np.float32(col),) + line(b, m_prev, s0))
                rslots = []
                for col, n in sorted(right):
                    m_at = seg(b, s0 + col)
                    rslots.append((np.float32(col - HALF),)
                                  + line(b, m_at, s0 + HALF))
                core_rows.append((baseL, baseR, lslots, rslots))
        per_row.append(core_rows)

    n_left = max(n_left, 1)
    n_right = max(n_right, 1)
    ncol = 3 + 3 * (n_left + n_right)
    rows = _prm_rows(n_left, n_right)
    pTs = []
    for c in range(NCORES):
        arr = np.zeros((rows, P + rows), dtype=np.float32)
        for p, (baseL, baseR, lslots, rslots) in enumerate(per_row[c]):
            vals = [baseL[0], baseL[1], baseR]
            for j in range(n_left):
                vals += list(lslots[j]) if j < len(lslots) else [-1.0, 0.0, 0.0]
            for j in range(n_right):
                vals += list(rslots[j]) if j < len(rslots) else [4096.0, 0.0, 0.0]
            arr[:len(vals), p] = vals
        arr[:, P:] = np.eye(rows, dtype=np.float32)  # identity for PE transpose
        pTs.append(arr)
    return pTs, (n_left, n_right)


def _prm_rows(n_left, n_right):
    """Partition rows of the transposed parameter block (even, = used cols)."""
    ncol = 3 + 3 * (n_left + n_right)
    rows = ncol + (ncol & 1)
    assert rows <= 128, f"too many breakpoint slots for one row: {ncol}"
    return rows


def _build(n_left, n_right):
    import concourse.bacc as bacc
    import concourse.mybir as mybir
    from concourse.tile import TileContext

    f32 = mybir.dt.float32
    Alu = mybir.AluOpType
    Act = mybir.ActivationFunctionType
    rows = _prm_rows(n_left, n_right)

    nc = bacc.Bacc("TRN2", debug=False, enable_asserts=False,
                   enable_partition_id=False, monotonic_sem_count=0)
    pT_dram = nc.dram_tensor("pT", [rows, P + rows], f32,
                             kind="ExternalInput").ap()
    y_dram = nc.dram_tensor("y", [P, W], f32, kind="ExternalOutput").ap()

    with TileContext(nc) as tc:
        with tc.tile_pool(name="pool", bufs=1) as pool, \
             tc.tile_pool(name="psum", bufs=1, space="PSUM") as psum_pool:
            # params arrive transposed ([32, 128]: 32 big descriptors instead
            # of 128 tiny ones); the idle PE un-transposes them via an
            # identity matmul.
            pT = pool.tile([rows, P + rows], f32, name="pT_t", tag="pT_t")
            nc.sync.dma_start(out=pT[:], in_=pT_dram[:])
            prm_ps = psum_pool.tile([P, rows], f32, name="prm_ps", tag="prm_ps")
            nc.tensor.transpose(prm_ps[:], pT[:, :P], pT[:, P:])
            prm = pool.tile([P, rows], f32, name="prm", tag="prm")
            nc.vector.tensor_copy(out=prm[:], in_=prm_ps[:])

            def sc(j):  # scalar AP = params column j
                return prm[:, j:j + 1]

            # warm the activation table off the critical path
            warm = pool.tile([P, 2], f32, name="warm", tag="warm")
            nc.vector.memset(warm[:], 0.0)
            nc.scalar.activation(warm[:, 1:2], warm[:, 0:1], Act.Identity)

            # local column index 0..HALF-1, shared by both halves (the host
            # expresses the right half in its own local coordinates)
            io = pool.tile([P, HALF], f32, name="io", tag="io")
            nc.gpsimd.iota(io[:], pattern=[[1, HALF]], base=0,
                           channel_multiplier=0,
                           allow_small_or_imprecise_dtypes=True)
            t = io[:]

            for h in range(2):
                o = pool.tile([P, HALF], f32, name=f"o{h}", tag=f"o{h}")
                # base line: o = f*aB + bB  (Scalar engine, fused FMA)
                nc.scalar.activation(o[:], t, Act.Identity,
                                     bias=sc(1 + h), scale=sc(0))
                if h == 0:
                    slots = [(3 + 3 * j, Alu.is_lt) for j in range(n_left)]
                else:
                    slots = [(3 + 3 * (n_left + j), Alu.is_ge)
                             for j in range(n_right)]
                for si, (bc, cmp_op) in enumerate(slots):
                    m = pool.tile([P, HALF], mybir.dt.uint8,
                                  name=f"m{h}{si}", tag=f"m{h}{si}")
                    ln = pool.tile([P, HALF], f32,
                                   name=f"l{h}{si}", tag=f"l{h}{si}")
                    # masks on DVE, lines split ACT/DVE
                    nc.vector.tensor_scalar(m[:], t, sc(bc), None, cmp_op)
                    if h == 0:
                        nc.vector.tensor_scalar(ln[:], t, sc(bc + 1),
                                                sc(bc + 2), Alu.mult, Alu.add)
                    else:
                        nc.scalar.activation(ln[:], t, Act.Identity,
                                             bias=sc(bc + 2), scale=sc(bc + 1))
                    # split the merges so stores can start sooner
                    nc.vector.copy_predicated(o[:, :576], m[:, :576],
                                              ln[:, :576])
                    nc.vector.copy_predicated(o[:, 576:], m[:, 576:],
                                              ln[:, 576:])
                # stores: balance the two HWDGE queues (384 KB each)
                c0 = h * HALF
                if h == 0:
                    nc.sync.dma_start(out=y_dram[:, c0:c0 + 576],
                                      in_=o[:, :576])
                    nc.scalar.dma_start(out=y_dram[:, c0 + 576:c0 + HALF],
                                        in_=o[:, 576:])
                else:
                    nc.scalar.dma_start(out=y_dram[:, c0:c0 + 576],
                                        in_=o[:, :576])
                    nc.sync.dma_start(out=y_dram[:, c0 + 576:c0 + HALF],
                                      in_=o[:, 576:])

    nc.compile()
    return nc


def _get_compiled(n_left, n_right):
    key = (n_left, n_right)
    if key not in _compiled:
        _compiled[key] = _build(n_left, n_right)
    return _compiled[key]


def kernel(segment_x, segment_y):
    from concourse.bass_utils import run_bass_kernel_spmd

    pTs, (n_left, n_right) = _host_prep(segment_x, segment_y)
    nc = _get_compiled(n_left, n_right)
    in_maps = [{"pT": pTs[c]} for c in range(NCORES)]
    res = run_bass_kernel_spmd(nc, in_maps, core_ids=list(range(NCORES)))

    out = np.empty((B, S), dtype=np.float32)
    for c in range(NCORES):
        yc = res.results[c]["y"]  # [128, 1536]
        base = c * PIX_PER_CORE
        out[:, base:base + PIX_PER_CORE] = yc.reshape(B, RPB * W)
    return out
